# revision 1
# baseline (speedup 1.0000x reference)
"""Trainium2 Bass kernel for a prenorm transformer Block (B=8, N=1024, D=768,
12 heads, MLP hidden 3072), data-parallel over batch across 8 NeuronCores.

v2 rewrite of the baseline. Same transposed layout (features on partitions,
tokens on free dim), restructured for engine balance:

  - Attention is scalar(exp)-bound: scores are row-tiled (even head contracts
    PE rows 0:63, odd head rows 64:127 via tile_position) so no zero-padded
    k planes; one [128,1024] exp per (pair, head, token-tile).
  - Softmax denominators come out of the attn@v matmul via a ones-column on V
    (partition 64 of cps); recips are batched: den rows DMA-gather to a
    [6,1024] tile, one Ln + one Exp(scale=-1) on the scalar engine (both live
    in the natural_log_exp table set together with the softmax Exp -> zero
    ACT table switches in the whole attention+LN pipeline), then DMA
    partition-broadcast back. No DVE reciprocals (3.9us each) anywhere.
  - LayerNorm rsqrt = Exp(-0.5*Ln(var+eps)) -- same table set again. Only 3
    ACT table loads in the whole kernel (nl-exp, gelu, nl-exp).
  - All activations/residuals in fp16 (2x/4x DVE modes); input x is consumed
    as fp16 (residual quantization ~2e-4, far under the 2e-2 gate).
  - fc2 accumulates over all 24 hidden chunks in PSUM (start/stop groups)
    instead of 60 DVE partial-sum adds.
  - MLP runs nb-split with all of fc1 (both halves) before fc2 so the gelu
    table set loads once; LN2(nb0) + output DMA overlap fc2(nb1).
  - All weights prefetched: wqkv/x16 first, wproj/w1 during attention, w2
    during proj/fc1.
"""
import sys
import types

sys.path.insert(0, "/opt/trn_rl_repo")

try:
    import antenv.axon_hooks  # noqa: F401
except Exception:
    try:
        import antenv

        _hooks = types.ModuleType("antenv.axon_hooks")
        _hooks._hook = None

        def _set_hook(h):
            _hooks._hook = h

        def _get_hook():
            return _hooks._hook

        _hooks.set_axon_ntff_profile_hook = _set_hook
        _hooks.get_axon_ntff_profile_hook = _get_hook
        sys.modules["antenv.axon_hooks"] = _hooks
        antenv.axon_hooks = _hooks
    except Exception:
        pass

import ml_dtypes
import numpy as np

import concourse.bass as bass
import concourse.tile as tile
from concourse import mybir
from concourse.bass_utils import run_bass_kernel_spmd

F32R = mybir.dt.float32r
F32 = mybir.dt.float32
F16 = mybir.dt.float16
F8 = mybir.dt.float8e4
DR = mybir.MatmulPerfMode.DoubleRow
AF = mybir.ActivationFunctionType
OP = mybir.AluOpType
XS, WS, VS = 16.0, 256.0, 8.0        # fp8 scales: x, weights, v

NCORES = 8
D, HEADS, HID, N = 768, 12, 3072, 1024
HD = D // HEADS                  # 64 head dim
DC = D // 128                    # 6 feature chunks
NB = N // 512                    # 2 moving-dim blocks
MT = N // 128                    # 8 token tiles
NPR = HEADS // 2                 # 6 head pairs
EPS = 1e-6

LAST_RESULT = None


# The walrus build in this container rejects instructions carrying more than
# a couple of sync waits; hoist excess waits onto standalone EventSemaphore
# carriers on the same engine (semantically identical).
_MM_OPS = ("Matmult", "Ldweights")


def _split_excess_waits(nc, default_limit=1, matmul_limit=0):
    counter = 0
    for f in nc.m.functions:
        for bb in f.blocks:
            new_insts = []
            for inst in bb.instructions:
                si = inst.sync_info
                waits = list(si.on_wait) if si and si.on_wait else []
                limit = matmul_limit if inst.opcode in _MM_OPS else default_limit
                if len(waits) > limit:
                    keep, move = waits[:limit], waits[limit:]
                    for w in move:
                        counter += 1
                        ev = mybir.InstEventSemaphore(
                            name=f"I-waitsplit-{counter}",
                            engine=inst.engine,
                            sync_info=mybir.SyncInfo(on_wait=[w], on_update=[]),
                        )
                        nc.register_instruction(ev, overwrite=True)
                        new_insts.append(ev)
                    inst.sync_info = mybir.SyncInfo(
                        on_wait=keep, on_update=list(si.on_update) if si else []
                    )
                new_insts.append(inst)
            bb.instructions = new_insts
    return counter


def _build(trivial_affine=False):
    nc = bass.Bass()

    xT16 = nc.dram_tensor("xT16", [D, N], F16, kind="ExternalInput")
    xT8 = nc.dram_tensor("xT8", [D, N], F8, kind="ExternalInput")
    wqkvT = nc.dram_tensor("wqkvT", [D, 3 * D], F8, kind="ExternalInput")
    wprojT = nc.dram_tensor("wprojT", [D, D], F16, kind="ExternalInput")
    wfc1T = nc.dram_tensor("wfc1T", [D, HID], F16, kind="ExternalInput")
    wfc2T = nc.dram_tensor("wfc2T", [HID, D], F8, kind="ExternalInput")
    bprojC = nc.dram_tensor("bprojC", [128, DC], F32, kind="ExternalInput")
    bfc1C = nc.dram_tensor("bfc1C", [128, HID // 128], F32, kind="ExternalInput")
    bfc2C = nc.dram_tensor("bfc2C", [128, DC], F32, kind="ExternalInput")
    gamma1C = nc.dram_tensor("gamma1C", [128, DC], F32, kind="ExternalInput")
    beta1C = nc.dram_tensor("beta1C", [128, DC], F32, kind="ExternalInput")
    gamma2C = nc.dram_tensor("gamma2C", [128, DC], F32, kind="ExternalInput")
    beta2C = nc.dram_tensor("beta2C", [128, DC], F32, kind="ExternalInput")
    yT = nc.dram_tensor("yT", [D, N], F32, kind="ExternalOutput")

    with tile.TileContext(nc) as tc:
        # ---- long-lived left-side pools (pushed first, released last) ----
        const = tc.alloc_tile_pool(name="const", bufs=1)
        stats = tc.alloc_tile_pool(name="stats", bufs=1)
        p_x1 = tc.alloc_tile_pool(name="p_x1", bufs=1)
        p_sq = tc.alloc_tile_pool(name="p_sq", bufs=1)
        p_u = tc.alloc_tile_pool(name="p_u", bufs=2)
        dscr = tc.alloc_tile_pool(name="dscr", bufs=1, space="DRAM")

        ones16 = const.tile([128, 1], F16)
        nc.vector.tensor_copy(ones16[:], nc.const_aps.tensor(1.0, (128, 1)))
        onesrow = const.tile([1, 128], F32R)
        nc.vector.tensor_copy(onesrow[:], nc.const_aps.tensor(1.0, (1, 128)))
        eps_t = const.tile([1, 1], F32)
        nc.vector.memset(eps_t[:], EPS)
        bproj_sb = const.tile([128, DC], F32)
        bfc1_sb = const.tile([128, HID // 128], F32)
        bfc2_sb = const.tile([128, DC], F32)
        g1_sb = const.tile([128, DC], F32)
        b1_sb = const.tile([128, DC], F32)
        g2_sb = const.tile([128, DC], F32)
        b2_sb = const.tile([128, DC], F32)
        x1_sb = p_x1.tile([128, DC, N], F16)
        warm = stats.tile([1, 8], F32, tag="warm", name="warm")
        nc.vector.memset(warm[:], 1.0)
        nc.scalar.activation(out=warm[:], in_=warm[:], func=AF.Exp)

        # ------------- Phase 1: QKV projections -------------
        # x + v-column weights first: the v matmuls are the kernel's first
        # tensor work, so these DMAs gate the startup stall.
        p_x16 = tc.alloc_tile_pool(name="p_x16", bufs=1)
        x16_sb = p_x16.tile([128, DC, N], F16)
        x8_sb = p_x16.tile([128, DC, N], F8)
        p_wqkv = tc.alloc_tile_pool(name="p_wqkv", bufs=1)
        wqkv_sb = p_wqkv.tile([128, DC, 3 * D], F8)
        for c in range(DC):
            for half in range(2):
                hs = slice(half * 512, half * 512 + 512)
                nc.sync.dma_start(out=x8_sb[:, c, hs],
                                  in_=xT8[c * 128:(c + 1) * 128, hs])
        for c in range(DC):
            for half in range(2):
                hs = slice(2 * D + half * 384, 2 * D + half * 384 + 384)
                nc.sync.dma_start(out=wqkv_sb[:, c, hs],
                                  in_=wqkvT[c * 128:(c + 1) * 128, hs])
        for c in range(DC):
            nc.sync.dma_start(out=wqkv_sb[:, c, 0:D],
                              in_=wqkvT[c * 128:(c + 1) * 128, 0:D])
        for c in range(DC):
            nc.sync.dma_start(out=wqkv_sb[:, c, D:2 * D],
                              in_=wqkvT[c * 128:(c + 1) * 128, D:2 * D])
        for c in range(DC):
            nc.sync.dma_start(out=x16_sb[:, c, :], in_=xT16[c * 128:(c + 1) * 128, :])
        for t, src in ((bproj_sb, bprojC), (bfc1_sb, bfc1C), (bfc2_sb, bfc2C),
                       (g1_sb, gamma1C), (b1_sb, beta1C), (g2_sb, gamma2C),
                       (b2_sb, beta2C)):
            nc.sync.dma_start(out=t[:], in_=src[:])

        p_qk = tc.alloc_tile_pool(name="p_qk", bufs=1, side="right")
        p_v = tc.alloc_tile_pool(name="p_v", bufs=1, side="right")
        q_sb = p_qk.tile([128, DC, N], F16)
        k2_sb = p_qk.tile([128, 2 * DC, N], F16)
        nc.vector.memset(k2_sb[64:128, 0:DC, :], 0.0)
        nc.vector.memset(k2_sb[0:64, DC:2 * DC, :], 0.0)
        v_sb = p_v.tile([128, MT, HEADS, HD + 1], F16)
        nc.vector.memset(v_sb[:, :, :, HD:HD + 1], 1.0)

        # v in direct layout: [token (partitions), v-dim]
        ps_v = tc.alloc_tile_pool(name="ps_v", bufs=2, space="PSUM")
        for mt in range(MT):
            msl = slice(mt * 128, mt * 128 + 128)
            ps = ps_v.tile([128, D], F32, tag="v", name="psv")
            for j in range(DC // 2):
                nc.tensor.matmul(ps[:, 0:512], x8_sb[:, 2 * j:2 * j + 2, msl],
                                 wqkv_sb[:, 2 * j:2 * j + 2, 2 * D:2 * D + 512],
                                 start=(j == 0), stop=(j == DC // 2 - 1),
                                 perf_mode=DR)
                nc.tensor.matmul(ps[:, 512:768], x8_sb[:, 2 * j:2 * j + 2, msl],
                                 wqkv_sb[:, 2 * j:2 * j + 2, 2 * D + 512:3 * D],
                                 start=(j == 0), stop=(j == DC // 2 - 1),
                                 perf_mode=DR)
            nc.vector.tensor_scalar_mul(
                v_sb[:, mt, :, 0:HD],
                in0=ps[:].rearrange("p (h d) -> p h d", h=HEADS),
                scalar1=1.0 / (XS * WS))
        ps_v.release()

        # scores psum allocated BEFORE the qk pool so the first score
        # matmuls don't wait for the whole qk-phase psum to drain
        ps_s = tc.alloc_tile_pool(name="ps_s", bufs=1, space="PSUM")
        # q,k transposed: [qkv-row tile (partitions), tokens]
        ps_qk = tc.alloc_tile_pool(name="ps_qk", bufs=4, space="PSUM")
        for jt in [x for p in range(DC) for x in (p, DC + p)]:
            pr = jt % DC
            col0 = jt * 128
            for nb in range(NB):
                sl = slice(nb * 512, nb * 512 + 512)
                ps = ps_qk.tile([128, 512], F32, tag="qk", name="psqk")
                for j in range(DC // 2):
                    nc.tensor.matmul(ps[:], wqkv_sb[:, 2 * j:2 * j + 2, col0:col0 + 128],
                                     x8_sb[:, 2 * j:2 * j + 2, sl],
                                     start=(j == 0), stop=(j == DC // 2 - 1),
                                     perf_mode=DR)
                if jt < DC:
                    nc.vector.tensor_scalar_mul(q_sb[:, pr, sl], in0=ps[:],
                                                scalar1=1.0 / (XS * WS))
                else:
                    nc.vector.tensor_scalar_mul(k2_sb[0:64, pr, sl],
                                                in0=ps[0:64, :],
                                                scalar1=1.0 / (XS * WS))
                    nc.vector.tensor_scalar_mul(k2_sb[64:128, DC + pr, sl],
                                                in0=ps[64:128, :],
                                                scalar1=1.0 / (XS * WS))
        ps_qk.release()
        p_wqkv.release()

        # prefetch proj weights while attention runs
        p_wproj = tc.alloc_tile_pool(name="p_wproj", bufs=1)
        wproj_sb = p_wproj.tile([128, DC, D], F16)
        for c in range(DC):
            nc.sync.dma_start(out=wproj_sb[:, c, :], in_=wprojT[c * 128:(c + 1) * 128, :])

        # ------------- Phase 2: attention -------------
        p_ctx = tc.alloc_tile_pool(name="p_ctx", bufs=1)
        ctx_sb = p_ctx.tile([128, DC, N], F16)
        p_ae = tc.alloc_tile_pool(name="p_ae", bufs=4, side="right")
        p_craw = tc.alloc_tile_pool(name="p_craw", bufs=1, side="right")
        p_recb = tc.alloc_tile_pool(name="p_recb", bufs=2, side="right")
        p_dg = tc.alloc_tile_pool(name="p_dg", bufs=2, side="right")
        ps_c = tc.alloc_tile_pool(name="ps_c", bufs=1, space="PSUM")

        den_d = dscr.tile([HEADS, N], F16, tag="den_d", name="den_d")
        craws = {}

        GROUPS = {2: (0, 6), 4: (6, 10), 5: (10, 12)}

        def den_group(g):
            """Batch-reciprocal denominators for a group of heads and
            normalize their context: one Ln + one Exp (same ACT table set
            as the softmax exp) instead of per-head reciprocals. The last
            group is a single head pair to shorten the attention->proj
            transition tail."""
            h0, h1 = GROUPS[g]
            nh = h1 - h0
            dg = p_dg.tile([6, N], F16, tag="dg", name="dg")
            nc.sync.dma_start(out=dg[0:nh, :], in_=den_d[h0:h1, :])
            lng = stats.tile([6, N], F16, tag="lng", name="lng")
            nc.scalar.activation(out=lng[0:nh, :], in_=dg[0:nh, :], func=AF.Ln)
            rec = stats.tile([6, N], F16, tag="rec", name="rec")
            nc.scalar.activation(out=rec[0:nh, :], in_=lng[0:nh, :], func=AF.Exp,
                                 scale=-1.0)
            rec_d = dscr.tile([6, N], F16, tag=f"recd{g}", name="rec_d")
            nc.sync.dma_start(out=rec_d[0:nh, :], in_=rec[0:nh, :])
            for i in range(nh):
                h = h0 + i
                prh, h01 = h // 2, h % 2
                half = h01 * 64
                recb = p_recb.tile([64, N], F16, tag="recb", name="recb")
                nc.sync.dma_start(
                    out=recb[:],
                    in_=rec_d[i:i + 1, :].to_broadcast([64, N]))
                nc.vector.tensor_mul(ctx_sb[half:half + 64, prh, :],
                                     craws[h][0:HD, :], recb[:])

        cps = {}

        def emit_av(pr, mt, ae, ao):
            """attn@v for one (pr, mt) step; emitted one step late so the
            PE's in-order stream never stalls on the exp it depends on. The
            cps accumulators are (re)allocated here at mt==0, safely after
            the previous pair's craw copies were emitted. On the last token
            tile also fold in the per-pair epilogue."""
            if mt == 0:
                for h01 in range(2):
                    for nb in range(NB):
                        cps[(h01, nb)] = ps_c.tile(
                            [HD + 1, 512], F32,
                            tag=f"c{h01}{nb}", name=f"cps{h01}{nb}")
            for h01, at_t in ((0, ae), (1, ao)):
                h = 2 * pr + h01
                for nb in range(NB):
                    sl = slice(nb * 512, nb * 512 + 512)
                    nc.tensor.matmul(cps[(h01, nb)][:], v_sb[:, mt, h, :],
                                     at_t[:, sl],
                                     start=(mt == 0), stop=(mt == MT - 1))
            if mt == MT - 1:
                for h01 in range(2):
                    h = 2 * pr + h01
                    craw = p_craw.tile([HD + 1, N], F16, tag=f"cr{h}", name="craw")
                    for nb in range(NB):
                        sl = slice(nb * 512, nb * 512 + 512)
                        nc.vector.tensor_copy(craw[:, sl], cps[(h01, nb)][:])
                    craws[h] = craw
                    nc.sync.dma_start(out=den_d[h:h + 1, :],
                                      in_=craw[HD:HD + 1, :])
                if pr in (2, 4, 5):
                    den_group(pr)

        pending = None
        for pr in range(NPR):
            for mt in range(MT):
                msl = slice(mt * 128, mt * 128 + 128)
                pse = ps_s.tile([128, N], F32, tag="pse", name="pse")
                pso = ps_s.tile([128, N], F32, tag="pso", name="pso")
                for nb in range(NB):
                    sl = slice(nb * 512, nb * 512 + 512)
                    nc.tensor.matmul(pse[:, sl], k2_sb[:, pr, msl],
                                     q_sb[:, pr, sl], start=True, stop=True)
                    nc.tensor.matmul(pso[:, sl], k2_sb[:, DC + pr, msl],
                                     q_sb[:, pr, sl], start=True, stop=True)
                ae = p_ae.tile([128, N], F16, tag="attnT", name="ae")
                ao = p_ae.tile([128, N], F16, tag="attnT", name="ao")
                nc.scalar.activation(out=ae[:], in_=pse[:], func=AF.Exp)
                nc.scalar.activation(out=ao[:], in_=pso[:], func=AF.Exp)
                if pending is not None:
                    emit_av(*pending)
                pending = (pr, mt, ae, ao)
        emit_av(*pending)
        ps_c.release()
        ps_s.release()
        p_dg.release()
        p_recb.release()
        p_craw.release()
        p_ae.release()
        p_v.release()
        p_qk.release()

        # prefetch MLP weights during proj/LN1/fc1
        p_w2 = tc.alloc_tile_pool(name="p_w2", bufs=1, side="right")
        w2_sb = p_w2.tile([128, HID // 128, D], F8)
        p_w1 = tc.alloc_tile_pool(name="p_w1", bufs=1, side="right")
        w1_sb = p_w1.tile([128, DC, HID], F16)
        for c in range(DC):
            nc.sync.dma_start(out=w1_sb[:, c, :], in_=wfc1T[c * 128:(c + 1) * 128, :])
        for fc in range(HID // 128):
            nc.sync.dma_start(out=w2_sb[:, fc, :], in_=wfc2T[fc * 128:(fc + 1) * 128, :])

        # ------------- Phase 3: proj + residual + LN1 -------------
        p_r1 = tc.alloc_tile_pool(name="p_r1", bufs=1)
        r1_sb = p_r1.tile([128, DC, N], F16)
        ps_ln = tc.alloc_tile_pool(name="ps_ln", bufs=1, space="PSUM")
        ps_ab = tc.alloc_tile_pool(name="ps_ab", bufs=1, space="PSUM")
        # one shared 4-deep pool for proj/fc1/fc2 output tiles: no PSUM
        # zone hand-offs between the back-half phases
        ps_mm = tc.alloc_tile_pool(name="ps_mm", bufs=4, space="PSUM")

        def ln_accum(src_sb, et, sl, s1, s2, first, last):
            """Fold chunk et of the pre-norm tensor into the LN sums. The
            square runs on GPSIMD (otherwise idle) for half the chunks."""
            sqt = p_u.tile([128, 512], F16, tag="sqt", name="sqt")
            eng = nc.gpsimd if et in (1, 4) else nc.vector
            eng.tensor_mul(sqt[:], src_sb[:, et, sl], src_sb[:, et, sl])
            nc.tensor.matmul(s1[:], ones16[:], src_sb[:, et, sl],
                             start=first, stop=last)
            nc.tensor.matmul(s2[:], ones16[:], sqt[:], start=first, stop=last)

        def ln_finish(src_sb, nb, s1, s2, gam, bet, out_sb, out_f32_sb=None,
                      out_dram=None):
            """Stats chain + affine for token half nb. rsqrt(var+eps) is
            Exp(-0.5*Ln(.)) so everything stays in the nl-exp ACT set."""
            sl = slice(nb * 512, nb * 512 + 512)
            t0 = stats.tile([1, 512], F32, tag="t0", name="t0")   # -mu
            t2 = stats.tile([1, 512], F32, tag="t2", name="t2")
            t4 = stats.tile([1, 512], F32, tag="t4", name="t4")
            t3 = stats.tile([1, 512], F32R, tag="t3", name="t3")  # a=1/std
            t1 = stats.tile([1, 512], F32R, tag="t1", name="t1")  # b=-mu/std
            nc.scalar.activation(out=t0[:], in_=s1[:], func=AF.Copy, scale=-1.0 / D)
            nc.scalar.activation(out=t4[:], in_=s1[:], func=AF.Square, scale=1.0 / D)
            nc.scalar.activation(out=t2[:], in_=s2[:], func=AF.Copy, scale=1.0 / D)
            nc.vector.tensor_sub(t2[:], t2[:], t4[:])            # var
            nc.scalar.activation(out=t4[:], in_=t2[:], func=AF.Ln, bias=eps_t[:])
            nc.scalar.activation(out=t3[:], in_=t4[:], func=AF.Exp,
                                 scale=-0.5)                     # rsqrt(var+eps)
            nc.vector.tensor_mul(t1[:], t0[:], t3[:].bitcast(F32))
            abp = ps_ab.tile([128, 2, 512], F32, tag="abp", name="abp")
            nc.tensor.matmul(abp[:, 0, :], onesrow[:], t3[:], start=True, stop=True)
            nc.tensor.matmul(abp[:, 1, :], onesrow[:], t1[:], start=True, stop=True)
            abps = stats.tile([128, 2, 512], F16, tag=f"abps{nb}", name="abps")
            nc.vector.tensor_copy(abps[:], abp[:])
            if trivial_affine:
                # gamma==1, beta==0: out = src*a + b for all 6 chunks in two
                # fused DVE ops using zero-stride broadcast of the per-token
                # scale/shift rows
                ab0 = abps[:, 0:1, :].to_broadcast([128, DC, 512])
                ab1 = abps[:, 1:2, :].to_broadcast([128, DC, 512])
                uall = p_sq.tile([128, DC, 512], F16, tag="uall", name="uall")
                nc.vector.tensor_mul(uall[:], src_sb[:, :, sl], ab0)
                if out_f32_sb is not None:
                    nc.vector.tensor_add(out_f32_sb[:], uall[:], ab1)
                    if out_dram is not None:
                        for c in range(DC):
                            nc.sync.dma_start(
                                out=out_dram[c * 128:(c + 1) * 128, sl],
                                in_=out_f32_sb[:, c, :])
                else:
                    nc.vector.tensor_add(out_sb[:, :, sl], uall[:], ab1)
                return
            for c in range(DC):
                u = p_u.tile([128, 512], F16, tag="u", name="u")
                nc.vector.tensor_mul(u[:], src_sb[:, c, sl], abps[:, 0, :])
                nc.vector.tensor_add(u[:], u[:], abps[:, 1, :])
                if out_f32_sb is not None:
                    nc.vector.tensor_scalar(out=out_f32_sb[:, c, :], in0=u[:],
                                            scalar1=gam[:, c:c + 1],
                                            scalar2=bet[:, c:c + 1],
                                            op0=OP.mult, op1=OP.add)
                    if out_dram is not None:
                        nc.sync.dma_start(out=out_dram[c * 128:(c + 1) * 128, sl],
                                          in_=out_f32_sb[:, c, :])
                else:
                    nc.vector.tensor_scalar(out=out_sb[:, c, sl], in0=u[:],
                                            scalar1=gam[:, c:c + 1],
                                            scalar2=bet[:, c:c + 1],
                                            op0=OP.mult, op1=OP.add)

        for nb in range(NB):
            sl = slice(nb * 512, nb * 512 + 512)
            s1 = ps_ln.tile([1, 512], F32, tag="s1", name="s1")
            s2 = ps_ln.tile([1, 512], F32, tag="s2", name="s2")
            for et in range(DC):
                ps = ps_mm.tile([128, 512], F32, tag="mm", name="pspj")
                for c in range(DC):
                    nc.tensor.matmul(ps[:], wproj_sb[:, c, et * 128:(et + 1) * 128],
                                     ctx_sb[:, c, sl],
                                     start=(c == 0), stop=(c == DC - 1))
                t = p_u.tile([128, 512], F16, tag="pt", name="pt")
                nc.scalar.activation(out=t[:], in_=ps[:], func=AF.Identity,
                                     bias=bproj_sb[:, et:et + 1], scale=1.0)
                nc.vector.tensor_add(r1_sb[:, et, sl], t[:], x16_sb[:, et, sl])
                if et > 0:
                    ln_accum(r1_sb, et - 1, sl, s1, s2, et - 1 == 0, False)
            ln_accum(r1_sb, DC - 1, sl, s1, s2, False, True)
            ln_finish(r1_sb, nb, s1, s2, g1_sb, b1_sb, x1_sb)
        p_r1.release()
        p_ctx.release()
        p_wproj.release()
        p_x16.release()

        # ------------- Phase 4: MLP (fc1 both halves, then fc2) -------------
        p_y2 = tc.alloc_tile_pool(name="p_y2", bufs=1)
        p_x2 = tc.alloc_tile_pool(name="p_x2", bufs=2)
        p_h = tc.alloc_tile_pool(name="p_h", bufs=1)
        y2_sb = p_y2.tile([128, DC, N], F16)
        h_sb = p_h.tile([128, HID // 128, N], F8)
        for nb in range(NB):
            sl = slice(nb * 512, nb * 512 + 512)
            for ftg in range(HID // 128):
                ps = ps_mm.tile([128, 512], F32, tag="mm", name="psf1")
                for c in range(DC):
                    nc.tensor.matmul(ps[:], w1_sb[:, c, ftg * 128:(ftg + 1) * 128],
                                     x1_sb[:, c, sl],
                                     start=(c == 0), stop=(c == DC - 1))
                nc.scalar.activation(out=h_sb[:, ftg, sl], in_=ps[:], func=AF.Gelu,
                                     bias=bfc1_sb[:, ftg:ftg + 1], scale=1.0)
        p_w1.release()

        for nb in range(NB):
            sl = slice(nb * 512, nb * 512 + 512)
            s1 = ps_ln.tile([1, 512], F32, tag="s1", name="s1")
            s2 = ps_ln.tile([1, 512], F32, tag="s2", name="s2")
            for et in range(DC):
                ps = ps_mm.tile([128, 512], F32, tag="mm", name="psf2")
                for j in range(HID // 256):
                    nc.tensor.matmul(ps[:],
                                     w2_sb[:, 2 * j:2 * j + 2,
                                           et * 128:(et + 1) * 128],
                                     h_sb[:, 2 * j:2 * j + 2, sl],
                                     start=(j == 0), stop=(j == HID // 256 - 1),
                                     perf_mode=DR)
                t = p_u.tile([128, 512], F16, tag="ft", name="ft")
                nc.scalar.activation(out=t[:], in_=ps[:], func=AF.Identity,
                                     bias=bfc2_sb[:, et:et + 1], scale=1.0 / WS)
                nc.vector.tensor_add(y2_sb[:, et, sl], t[:], x1_sb[:, et, sl])
                if et > 0:
                    ln_accum(y2_sb, et - 1, sl, s1, s2, et - 1 == 0, False)
            ln_accum(y2_sb, DC - 1, sl, s1, s2, False, True)
            x2 = p_x2.tile([128, DC, 512], F32, tag="x2", name="x2")
            ln_finish(y2_sb, nb, s1, s2, g2_sb, b2_sb, None, out_f32_sb=x2,
                      out_dram=yT)
        ps_mm.release()
        ps_ab.release()
        ps_ln.release()
        p_w2.release()
        dscr.release()
        p_h.release()
        p_x2.release()
        p_y2.release()
        p_u.release()
        p_sq.release()
        p_x1.release()
        stats.release()
        const.release()
    return nc


_NC_CACHE = {}


def _get_nc(trivial_affine=False):
    nc = _NC_CACHE.get(trivial_affine)
    if nc is None:
        nc = _build(trivial_affine)
        _split_excess_waits(nc)
        _NC_CACHE[trivial_affine] = nc
    return nc


def kernel(x, w_qkv, w_proj, b_proj, w_fc1, b_fc1, w_fc2, b_fc2,
           gamma1, beta1, gamma2, beta2):
    global LAST_RESULT
    x = np.asarray(x, dtype=np.float32)
    w_qkv = np.asarray(w_qkv, dtype=np.float32)
    w_proj = np.asarray(w_proj, dtype=np.float32)
    b_proj = np.asarray(b_proj, dtype=np.float32)
    w_fc1 = np.asarray(w_fc1, dtype=np.float32)
    b_fc1 = np.asarray(b_fc1, dtype=np.float32)
    w_fc2 = np.asarray(w_fc2, dtype=np.float32)
    b_fc2 = np.asarray(b_fc2, dtype=np.float32)
    gamma1 = np.asarray(gamma1, dtype=np.float32)
    beta1 = np.asarray(beta1, dtype=np.float32)
    gamma2 = np.asarray(gamma2, dtype=np.float32)
    beta2 = np.asarray(beta2, dtype=np.float32)

    F8NP = ml_dtypes.float8_e4m3
    wqkv_scaled = w_qkv.copy()
    wqkv_scaled[:D] *= HD ** -0.5                  # fold attention scale into Q
    wqkvT = np.ascontiguousarray((wqkv_scaled.T * 256.0).astype(F8NP))
    wprojT = np.ascontiguousarray(w_proj.T.astype(np.float16))
    wfc1T = np.ascontiguousarray(w_fc1.T.astype(np.float16))
    wfc2T = np.ascontiguousarray((w_fc2.T * 256.0).astype(F8NP))

    def cols(v, nchunks):
        return np.ascontiguousarray(v.reshape(nchunks, 128).T)

    shared = {
        "wqkvT": wqkvT, "wprojT": wprojT, "wfc1T": wfc1T, "wfc2T": wfc2T,
        "bprojC": cols(b_proj, DC), "bfc1C": cols(b_fc1, HID // 128),
        "bfc2C": cols(b_fc2, DC),
        "gamma1C": cols(gamma1, DC), "beta1C": cols(beta1, DC),
        "gamma2C": cols(gamma2, DC), "beta2C": cols(beta2, DC),
    }
    in_maps = []
    for b in range(NCORES):
        m = dict(shared)
        xt = np.ascontiguousarray(x[b].T)
        m["xT16"] = xt.astype(np.float16)
        m["xT8"] = (xt * 16.0).astype(F8NP)
        in_maps.append(m)

    trivial = (np.all(gamma1 == 1.0) and np.all(beta1 == 0.0)
               and np.all(gamma2 == 1.0) and np.all(beta2 == 0.0))
    nc = _get_nc(trivial_affine=bool(trivial))
    LAST_RESULT = run_bass_kernel_spmd(nc, in_maps, list(range(NCORES)))
    out = np.stack([np.ascontiguousarray(LAST_RESULT.results[b]["yT"].T)
                    for b in range(NCORES)])
    return out.astype(np.float32)

